# revision 10
# baseline (speedup 1.0000x reference)
"""MFA block kernel for 8 Trainium2 NeuronCores.

Full (unsharded) inputs in, full output out.

Strategy (v2): ZERO collectives, no transposed-AP DMAs.  Probing showed
each gpsimd collective_compute costs ~5 ms on this stack and 4-byte
element-transposed DMAs cost ~1.3 ms each, dwarfing the ~100 us of real
work.  So instead of AllReducing the small Gram matrices, every core
receives the FULL inputs (in bf16) and computes the token-global Gram
matrices redundantly; only the per-token work (theta / w_y / z) is
sharded.  All cross-layout moves use tensor-engine transposes.

Math (associative rewrite, w_b dropped -- BN removes constant shifts):
    C   = X_ext^T X_ext            X_ext = [x_l | 1]     (257 x 257)
    M'  = P_ext^T C G_ext / BN                           (256 x 256)
    V   = M' @ w_w                                       (256 x 512)
    w_y = theta @ V,   theta = Xh_ext @ Th_ext           (per-token)
    BN stats WITHOUT reducing w_y over cores:
      Ch  = Xh_ext^T Xh_ext        Xh_ext = [x_h | 1]    (513 x 513)
      W   = Th_ext @ V                                   (513 x 512)
      sum_t w_y[t,f]   = (Ch row 512) @ W                (= s_xe @ W)
      sum_t w_y[t,f]^2 = diag(W^T Ch W) = colsum(W o (Ch @ W))
    z = (w_y - mean) * A + beta + x_h,  A = gamma*rsqrt(var+eps)
      = theta @ (V * A) + (beta - mean*A) + x_h

Each core's x_h copy is rolled so its own 1024 tokens are tiles 0..7
(Grams are permutation-invariant); the residual uses an fp32 local slice.
"""

import threading

import numpy as np

import concourse.tile as tile
from concourse import bacc, mybir
from concourse.bass_utils import run_bass_kernel_spmd

FP = mybir.dt.float32
BF = mybir.dt.bfloat16
HIGH = 512
LOW = 256
B = 8
N = 1024
BN = B * N            # 8192 flattened tokens
NCORES = 8
TPC = BN // NCORES    # 1024 tokens per core
NT = BN // 128        # 64 global token tiles
LT = TPC // 128       # 8 local token tiles
HC = HIGH // 128      # 4 feature chunks of x_h
LC = LOW // 128       # 2 feature chunks of low dim
EPS = 1e-5

LOWE = LOW + 1        # 257
HIGHE = HIGH + 1      # 513


def build_kernel(repeats: int = 1, noar: bool = False):
    nc = bacc.Bacc("TRN2", target_bir_lowering=False, debug=False,
                   num_devices=NCORES)

    xlb = nc.declare_dram_parameter("xlb", [BN, LOW], BF, isOutput=False)
    xhb = nc.declare_dram_parameter("xhb", [BN, HIGH], BF, isOutput=False)
    xh_loc = nc.declare_dram_parameter("xh_loc", [TPC, HIGH], FP,
                                       isOutput=False)
    thw = nc.declare_dram_parameter("thw", [HIGH, LOW], BF, isOutput=False)
    thwT = nc.declare_dram_parameter("thwT", [LOW, HIGH], BF, isOutput=False)
    thb = nc.declare_dram_parameter("thb", [LOW], FP, isOutput=False)
    thb_b = nc.declare_dram_parameter("thb_b", [LOW], BF, isOutput=False)
    gw = nc.declare_dram_parameter("gw", [LOW, LOW], BF, isOutput=False)
    gb_b = nc.declare_dram_parameter("gb_b", [LOW], BF, isOutput=False)
    pw = nc.declare_dram_parameter("pw", [LOW, LOW], BF, isOutput=False)
    pb_b = nc.declare_dram_parameter("pb_b", [LOW], BF, isOutput=False)
    ww = nc.declare_dram_parameter("ww", [LOW, HIGH], BF, isOutput=False)
    gamma = nc.declare_dram_parameter("gamma", [HIGH], FP, isOutput=False)
    beta = nc.declare_dram_parameter("beta", [HIGH], FP, isOutput=False)
    ident = nc.declare_dram_parameter("ident", [128, 128], BF, isOutput=False)
    z_out = nc.declare_dram_parameter("z", [TPC, HIGH], FP, isOutput=True)

    with tile.TileContext(nc) as tc:
        with (
            tc.tile_pool(name="sb", bufs=1) as sb,
            tc.tile_pool(name="ps", bufs=1, space="PSUM") as ps,
        ):
            # ---- constants
            eps_c = sb.tile([1, 1], FP, tag="eps_c")
            nc.vector.memset(eps_c, EPS)
            ones_r1 = sb.tile([1, 128], FP, tag="ones_r1")
            nc.vector.memset(ones_r1, 1.0)
            ones_cb = sb.tile([128, 1], BF, tag="ones_cb")
            nc.vector.memset(ones_cb, 1.0)
            bn_c = sb.tile([1, 1], BF, tag="bn_c")
            nc.vector.memset(bn_c, float(BN))
            one11 = sb.tile([1, 1], BF, tag="one11")
            nc.vector.memset(one11, 1.0)
            idn = sb.tile([128, 128], BF, tag="idn")
            nc.sync.dma_start(idn[:], ident[:, :])

            for _ in range(repeats):
                # ================= input DMAs =================
                xle = sb.tile([128, NT, LOWE], BF, tag="xle")
                nc.vector.memset(xle[:, :, LOW:LOWE], 1.0)
                for k in range(4):
                    nc.sync.dma_start(
                        xle[:, k * 16:(k + 1) * 16, 0:LOW],
                        xlb[k * 2048:(k + 1) * 2048, :].rearrange(
                            "(i p) f -> p i f", p=128))
                xhe = sb.tile([128, NT, HIGHE], BF, tag="xhe")
                nc.vector.memset(xhe[:, :, HIGH:HIGHE], 1.0)
                for k in range(8):
                    nc.sync.dma_start(
                        xhe[:, k * 8:(k + 1) * 8, 0:HIGH],
                        xhb[k * 1024:(k + 1) * 1024, :].rearrange(
                            "(i p) f -> p i f", p=128))

                # weights
                thf = sb.tile([128, HC, LOW], BF, tag="thf")
                nc.sync.dma_start(thf[:], thw[:, :].rearrange(
                    "(ko ki) a -> ki ko a", ki=128))
                thft = sb.tile([128, LC, HIGHE], BF, tag="thft")
                nc.sync.dma_start(thft[:, :, 0:HIGH], thwT[:, :].rearrange(
                    "(mc p) h -> p mc h", p=128))
                nc.sync.dma_start(
                    thft[:, :, HIGH:HIGHE],
                    thb_b[:].rearrange("(mc p) -> p mc", p=128)[:, :, None])
                thbp = sb.tile([128, LC], FP, tag="thbp")
                nc.sync.dma_start(thbp[:], thb[:].rearrange(
                    "(ko ki) -> ki ko", ki=128))
                gext = sb.tile([128, 3, LOW], BF, tag="gext")
                nc.sync.dma_start(gext[:, 0:2, :], gw[:, :].rearrange(
                    "(ko ki) a -> ki ko a", ki=128))
                nc.sync.dma_start(gext[0:1, 2, :], gb_b[:][None, :])
                pext = sb.tile([128, 3, LOW], BF, tag="pext")
                nc.sync.dma_start(pext[:, 0:2, :], pw[:, :].rearrange(
                    "(ko ki) a -> ki ko a", ki=128))
                nc.sync.dma_start(pext[0:1, 2, :], pb_b[:][None, :])
                wwb = sb.tile([128, LC, HIGH], BF, tag="wwb")
                nc.sync.dma_start(wwb[:], ww[:, :].rearrange(
                    "(ko ki) h -> ki ko h", ki=128))
                gamma_r = sb.tile([1, HIGH], FP, tag="gamma_r")
                nc.sync.dma_start(gamma_r[:], gamma[:][None, :])
                beta_r = sb.tile([1, HIGH], FP, tag="beta_r")
                nc.sync.dma_start(beta_r[:], beta[:][None, :])
                # fp32 local x_h for the residual
                xhl32 = sb.tile([128, LT, HIGH], FP, tag="xhl32")
                for k in range(2):
                    nc.sync.dma_start(
                        xhl32[:, k * 4:(k + 1) * 4, :],
                        xh_loc[k * 512:(k + 1) * 512, :].rearrange(
                            "(i p) f -> p i f", p=128))

                # ================= C = X_ext^T X_ext (257x257) =========
                cpf = [ps.tile([128, 512], FP, tag="acc", bufs=5,
                               name=f"cp{m}") for m in range(2)]
                cp = [t[:, 0:LOWE] for t in cpf]
                for i in range(NT):
                    for m in range(2):
                        nc.tensor.matmul(
                            cp[m], xle[:, i, m * 128:(m + 1) * 128],
                            xle[:, i, :], start=(i == 0), stop=(i == NT - 1))
                clb = sb.tile([128, 2, LOWE], BF, tag="clb")
                for m in range(2):
                    nc.vector.tensor_copy(clb[:, m, :], cp[m])
                # s row (row 256) from the s column, via tiny transposes
                srow = sb.tile([1, LOWE], BF, tag="srow")
                nc.vector.memset(srow[:, 256:LOWE], float(BN))
                for m in range(2):
                    spf = ps.tile([128, 512], BF, tag="tb", bufs=1,
                                  name="spf")
                    sp = spf[0:1, 0:128]
                    nc.tensor.transpose(sp, clb[:, m, 256:LOWE], idn[:])
                    nc.vector.tensor_copy(srow[:, m * 128:(m + 1) * 128], sp)

                # ================= Ch = Xh_ext^T Xh_ext (513x513) ======
                chp = [ps.tile([128, HIGH], FP, tag="acc", bufs=5,
                               name=f"chp{m}") for m in range(HC)]
                chsf = ps.tile([128, HIGH], FP, tag="acc", bufs=5)
                chs = chsf[0:1, :]
                for i in range(NT):
                    for m in range(HC):
                        nc.tensor.matmul(
                            chp[m], xhe[:, i, m * 128:(m + 1) * 128],
                            xhe[:, i, 0:HIGH], start=(i == 0),
                            stop=(i == NT - 1))
                    nc.tensor.matmul(
                        chs, xhe[:, i, HIGH:HIGHE], xhe[:, i, 0:HIGH],
                        start=(i == 0), stop=(i == NT - 1))
                chb = sb.tile([128, HC, HIGH], BF, tag="chb")
                for m in range(HC):
                    nc.vector.tensor_copy(chb[:, m, :], chp[m])
                sxe = sb.tile([1, HIGHE], BF, tag="sxe")
                nc.vector.tensor_copy(sxe[:, 0:HIGH], chs)
                nc.vector.memset(sxe[:, HIGH:HIGHE], float(BN))
                # s_x as columns [128, HC] for K-side use
                sxc = sb.tile([128, HC], BF, tag="sxc")
                for m in range(HC):
                    sxpf = ps.tile([128, 512], BF, tag="tb", bufs=1,
                                   name="sxpf")
                    sxp = sxpf[:, 0:1]
                    nc.tensor.transpose(
                        sxp, sxe[0:1, m * 128:(m + 1) * 128], idn[0:1, 0:1])
                    nc.vector.tensor_copy(sxc[:, m:m + 1], sxp)

                # ============ theta^T for local tokens (tiles 0..LT) ====
                xhtl = sb.tile([128, HC, TPC], BF, tag="xhtl")
                for j in range(LT):
                    tpp = ps.tile([128, 512], BF, tag="tb", bufs=1,
                                  name="tpp")
                    for h in range(HC):
                        nc.tensor.transpose(
                            tpp[:, h * 128:(h + 1) * 128],
                            xhe[:, j, h * 128:(h + 1) * 128], idn[:])
                    nc.vector.tensor_copy(
                        xhtl[:, :, j * 128:(j + 1) * 128],
                        tpp[:].rearrange("p (h q) -> p h q", h=HC))
                thtl = sb.tile([128, LC, TPC], BF, tag="thtl")
                for m in range(LC):
                    for s2 in range(TPC // 512):
                        tp = ps.tile([128, 512], FP, tag="mm", bufs=2)
                        for k in range(HC):
                            nc.tensor.matmul(
                                tp, thf[:, k, m * 128:(m + 1) * 128],
                                xhtl[:, k, s2 * 512:(s2 + 1) * 512],
                                start=(k == 0), stop=(k == HC - 1))
                        nc.vector.tensor_scalar(
                            thtl[:, m, s2 * 512:(s2 + 1) * 512], tp,
                            thbp[:, m:m + 1], None, mybir.AluOpType.add)

                # ============ T1 = C @ G_ext (257 x 256) ================
                def c_lhs(kc, msl):
                    # C_ext block [K rows kc, M cols msl] (C symmetric)
                    if kc < 2:
                        return clb[:, kc, msl]
                    return srow[0:1, msl]

                t1 = sb.tile([128, 2, LOW], BF, tag="t1")
                t1s = sb.tile([1, LOW], BF, tag="t1s")
                for m in range(3):
                    msl = (slice(0, 128), slice(128, 256),
                           slice(256, 257))[m]
                    mlen = msl.stop - msl.start
                    t1pf = ps.tile([128, 512], FP, tag="mm", bufs=2,
                                   name="t1pf")
                    t1p = t1pf[:, 0:LOW]
                    for k in range(3):
                        klen = 128 if k < 2 else 1
                        nc.tensor.matmul(
                            t1p[:mlen, :], c_lhs(k, msl),
                            gext[:klen, k, :], start=(k == 0), stop=(k == 2))
                    if m < 2:
                        nc.vector.tensor_copy(t1[:, m, :], t1p[:mlen, :])
                    else:
                        nc.vector.tensor_copy(t1s[:], t1p[:mlen, :])

                # ============ MpT = T1^T @ P_ext / BN = M'^T ===========
                mpt = sb.tile([128, LC, LOW], BF, tag="mpt")
                for m in range(LC):
                    mppf = ps.tile([128, 512], FP, tag="mm", bufs=2,
                                   name="mppf")
                    mpp = mppf[:, 0:LOW]
                    for k in range(3):
                        klen = 128 if k < 2 else 1
                        lhs = (t1[:, k, m * 128:(m + 1) * 128] if k < 2
                               else t1s[0:1, m * 128:(m + 1) * 128])
                        nc.tensor.matmul(mpp, lhs, pext[:klen, k, :],
                                         start=(k == 0), stop=(k == 2))
                    nc.vector.tensor_scalar_mul(mpt[:, m, :], mpp, 1.0 / BN)

                # ============ V = M' @ w_w (256 x 512) =================
                vsb = sb.tile([128, LC, HIGH], BF, tag="vsb")
                for m in range(LC):
                    vp = ps.tile([128, HIGH], FP, tag="mm", bufs=2)
                    for k in range(LC):
                        nc.tensor.matmul(
                            vp, mpt[:, k, m * 128:(m + 1) * 128],
                            wwb[:, k, :], start=(k == 0), stop=(k == LC - 1))
                    nc.vector.tensor_copy(vsb[:, m, :], vp)

                # ============ W = Th_ext @ V (513 x 512) ===============
                wsb = sb.tile([128, HC, HIGH], BF, tag="wsb")
                ws_row = sb.tile([1, HIGH], BF, tag="ws_row")
                for m in range(HC + 1):
                    msl = slice(m * 128, (m + 1) * 128) if m < HC else \
                        slice(HIGH, HIGHE)
                    mlen = 128 if m < HC else 1
                    wp = ps.tile([128, HIGH], FP, tag="mm", bufs=2)
                    for k in range(LC):
                        nc.tensor.matmul(wp[:mlen, :], thft[:, k, msl],
                                         vsb[:, k, :], start=(k == 0),
                                         stop=(k == LC - 1))
                    if m < HC:
                        nc.vector.tensor_copy(wsb[:, m, :], wp[:mlen, :])
                    else:
                        nc.vector.tensor_copy(ws_row[:], wp[:mlen, :])

                # ===== Z1 = Ch_ext @ W; d = colsum(W o Z1); s_w ========
                def ch_lhs(kc, m):
                    # Ch_ext block [K rows kc, M cols m] (Ch symmetric)
                    if kc < HC and m < HC:
                        return chb[:, kc, m * 128:(m + 1) * 128]
                    if kc == HC and m < HC:           # K row 512
                        return sxe[0:1, m * 128:(m + 1) * 128]
                    if kc < HC:                        # M col 512
                        return sxc[:, kc:kc + 1]
                    return bn_c[0:1, 0:1]

                def w_rhs(kc):
                    return wsb[:, kc, :] if kc < HC else ws_row[0:1, :]

                dpsf = ps.tile([128, HIGH], FP, tag="acc", bufs=5,
                               name="dpsf")
                dps = dpsf[0:1, :]
                for m in range(HC + 1):
                    mlen = 128 if m < HC else 1
                    z1p = ps.tile([128, HIGH], FP, tag="mm", bufs=2)
                    for k in range(HC + 1):
                        klen = 128 if k < HC else 1
                        nc.tensor.matmul(z1p[:mlen, :], ch_lhs(k, m),
                                         w_rhs(k), start=(k == 0),
                                         stop=(k == HC))
                    y1 = sb.tile([128, HIGH], BF, tag="y1", bufs=2)
                    nc.vector.tensor_mul(y1[:mlen, :], w_rhs(m),
                                         z1p[:mlen, :])
                    nc.tensor.matmul(
                        dps, ones_cb[:mlen, :] if m < HC else one11[0:1, 0:1],
                        y1[:mlen, :], start=(m == 0), stop=(m == HC))

                swpf = ps.tile([128, HIGH], FP, tag="acc", bufs=5,
                               name="swpf")
                swp = swpf[0:1, :]
                for k in range(HC + 1):
                    klen = 128 if k < HC else 1
                    lhs = sxc[:, k:k + 1] if k < HC else bn_c[0:1, 0:1]
                    nc.tensor.matmul(swp, lhs, w_rhs(k), start=(k == 0),
                                     stop=(k == HC))

                # ============ BN stats -> a_row / c_row (fp32) ==========
                mean_r = sb.tile([1, HIGH], FP, tag="mean_r")
                nc.vector.tensor_scalar_mul(mean_r[:], swp, 1.0 / BN)
                ex2_r = sb.tile([1, HIGH], FP, tag="ex2_r")
                nc.vector.tensor_scalar_mul(ex2_r[:], dps, 1.0 / BN)
                var_r = sb.tile([1, HIGH], FP, tag="var_r")
                nc.vector.tensor_mul(var_r[:], mean_r[:], mean_r[:])
                nc.vector.tensor_sub(var_r[:], ex2_r[:], var_r[:])
                rstd_r = sb.tile([1, HIGH], FP, tag="rstd_r")
                nc.scalar.activation(rstd_r[:], var_r[:],
                                     mybir.ActivationFunctionType.Sqrt,
                                     bias=eps_c[:])
                nc.vector.reciprocal(rstd_r[:], rstd_r[:])
                a_row = sb.tile([1, HIGH], FP, tag="a_row")
                nc.vector.tensor_mul(a_row[:], gamma_r[:], rstd_r[:])
                c_row = sb.tile([1, HIGH], FP, tag="c_row")
                nc.vector.tensor_mul(c_row[:], mean_r[:], a_row[:])
                nc.vector.tensor_sub(c_row[:], beta_r[:], c_row[:])

                # broadcast a/c over 128 partitions via K=1 matmul
                abp = ps.tile([128, HIGH], FP, tag="acc", bufs=5)
                nc.tensor.matmul(abp, ones_r1[:], a_row[:])
                a_bcb = sb.tile([128, HIGH], BF, tag="a_bcb")
                nc.vector.tensor_copy(a_bcb[:], abp)
                cbp = ps.tile([128, HIGH], FP, tag="acc", bufs=5)
                nc.tensor.matmul(cbp, ones_r1[:], c_row[:])
                c_bcf = sb.tile([128, HIGH], FP, tag="c_bcf")
                nc.vector.tensor_copy(c_bcf[:], cbp)

                # V' = V * a  (scale V columns)
                vpb = sb.tile([128, LC, HIGH], BF, tag="vpb")
                for m in range(LC):
                    nc.vector.tensor_mul(vpb[:, m, :], vsb[:, m, :],
                                         a_bcb[:])
                # r = x_h + c  (fp32)
                r_t = sb.tile([128, LT, HIGH], FP, tag="r_t")
                for j in range(LT):
                    nc.vector.tensor_add(r_t[:, j, :], xhl32[:, j, :],
                                         c_bcf[:])

                # ============ w_y + residual -> z (token-major) ========
                for j in range(LT):
                    wyp = ps.tile([128, HIGH], FP, tag="mm", bufs=2)
                    for k in range(LC):
                        nc.tensor.matmul(
                            wyp, thtl[:, k, j * 128:(j + 1) * 128],
                            vpb[:, k, :], start=(k == 0), stop=(k == LC - 1))
                    z_sb = sb.tile([128, HIGH], FP, tag="z_sb", bufs=2)
                    nc.vector.tensor_add(z_sb[:], wyp, r_t[:, j, :])
                    nc.sync.dma_start(
                        z_out[j * 128:(j + 1) * 128, :], z_sb[:])

    nc.compile()
    return nc


_CACHE: dict[int, "bacc.Bacc"] = {}
_LOCK = threading.Lock()


def _get_nc(repeats: int = 1):
    with _LOCK:
        if repeats not in _CACHE:
            _CACHE[repeats] = build_kernel(repeats)
        return _CACHE[repeats]


def _shard_inputs(inputs: dict) -> list[dict]:
    import ml_dtypes
    bf16 = ml_dtypes.bfloat16
    xh = np.asarray(inputs["x_h"], dtype=np.float32).reshape(BN, HIGH)
    xl = np.asarray(inputs["x_l"], dtype=np.float32).reshape(BN, LOW)
    xlb = np.ascontiguousarray(xl.astype(bf16))
    xh_b = xh.astype(bf16)
    thw = np.asarray(inputs["theta_w"], np.float32)
    common = {
        "xlb": xlb,
        "thw": np.ascontiguousarray(thw.astype(bf16)),
        "thwT": np.ascontiguousarray(thw.T.astype(bf16)),
        "thb": np.asarray(inputs["theta_b"], np.float32),
        "thb_b": np.asarray(inputs["theta_b"], np.float32).astype(bf16),
        "gw": np.ascontiguousarray(
            np.asarray(inputs["g_w"], np.float32).astype(bf16)),
        "gb_b": np.asarray(inputs["g_b"], np.float32).astype(bf16),
        "pw": np.ascontiguousarray(
            np.asarray(inputs["phi_w"], np.float32).astype(bf16)),
        "pb_b": np.asarray(inputs["phi_b"], np.float32).astype(bf16),
        "ww": np.ascontiguousarray(
            np.asarray(inputs["w_w"], np.float32).astype(bf16)),
        "gamma": np.asarray(inputs["bn_gamma"], np.float32),
        "beta": np.asarray(inputs["bn_beta"], np.float32),
        "ident": np.eye(128, dtype=bf16),
    }
    maps = []
    for c in range(NCORES):
        roll = np.ascontiguousarray(np.roll(xh_b, -c * TPC, axis=0))
        maps.append({
            "xhb": roll,
            "xh_loc": np.ascontiguousarray(xh[c * TPC:(c + 1) * TPC]),
            **common,
        })
    return maps


def kernel(**inputs) -> np.ndarray:
    nc = _get_nc(1)
    in_maps = _shard_inputs(inputs)
    res = run_bass_kernel_spmd(nc, in_maps, list(range(NCORES)))
    z = np.concatenate([res.results[c]["z"] for c in range(NCORES)], axis=0)
    return z.reshape(B, N, HIGH)


# revision 17
# speedup vs baseline: 11.9732x; 11.9732x over previous
"""MFA block kernel for 8 Trainium2 NeuronCores.

Full (unsharded) inputs in, full output out.

v4: tokens sharded 1024/core, TWO small AllReduces, minimal op count.
The execution stack here prices ops roughly as: matmul ~90us each
(shape-insensitive), vector op ~170us FIXED with near-free elements,
contiguous 2D DMA ~fast, strided DMA ~1ms/MB, AllReduce ~1-2ms.  So the
kernel minimizes op COUNT: Gram/matmul outputs are packed into large
PSUM regions drained by ONE big vector op each, stats use single huge
reduces, and all host-visible layouts are pre-tiled so every DMA is
per-partition contiguous.

Pipeline (everything bf16 except stats/residual math):
  C = X_ext^T X_ext (rect + s-row, 3 chained MM groups) -> AllReduce#1
  theta^T = Th^T x_h^T + thb  (feature-major, from host-shipped x_h^T)
  M' = P_ext^T C G_ext / BN;  V = M' @ w_w
  w_y^T = V^T theta^T         (feature-major)
  BN sums via one mul + two reduces -> AllReduce#2 ([128,8] f32)
  z^T = (w_y^T)*a + c + x_h^T   (a,c per-partition scalars), one DMA out
w_b is dropped (BN cancels constant shifts).
"""

import threading

import numpy as np

import concourse.tile as tile
from concourse import bacc, mybir
from concourse.bass_utils import run_bass_kernel_spmd

FP = mybir.dt.float32
BF = mybir.dt.bfloat16
HIGH = 512
LOW = 256
B = 8
N = 1024
BN = B * N
NCORES = 8
TPC = BN // NCORES    # 1024
LT = TPC // 128       # 8
HC = HIGH // 128      # 4
LC = LOW // 128       # 2
EPS = 1e-5
LOWE = LOW + 1        # 257

# weight blob column offsets (bf16, [128, WB])
O_THF = 0                      # [4, 256] theta_w (high-major)
O_PEXT = O_THF + 4 * 256       # [3, 256] P_ext chunks (row 256 on p0)
O_GWW = O_PEXT + 3 * 256       # [2, 512] Gww = G_ext @ w_w rows 0:512
O_GWWS = O_GWW + 2 * 512       # [1, 512] Gww row 256 (on p0)
WB = O_GWWS + 512

# C-region packing (psum cols, f32): m0 rows at 0, m1 rows at 512,
# s-row at 1024 (bank-aligned); payload ships cols 0:1536 raw.
CF = 1536

rg = [list(range(NCORES))]


def build_kernel(repeats: int = 1, noar: bool = False, stage: int = 9):
    nc = bacc.Bacc("TRN2", target_bir_lowering=False, debug=False,
                   num_devices=NCORES)

    xle_t = nc.declare_dram_parameter("xle_t", [128, LT * LOWE], BF,
                                      isOutput=False)
    xhtl_t = nc.declare_dram_parameter("xhtl_t", [128, HC * TPC], BF,
                                       isOutput=False)
    wblob = nc.declare_dram_parameter("wblob", [128, WB], BF, isOutput=False)
    gb_p = nc.declare_dram_parameter("gb_p", [128, 2 * HC], FP,
                                     isOutput=False)
    z_out = nc.declare_dram_parameter("z", [128, HC * TPC], FP,
                                      isOutput=True)

    with tile.TileContext(nc) as tc:
        with (
            tc.tile_pool(name="sb", bufs=1) as sb,
            tc.tile_pool(name="ps", bufs=1, space="PSUM") as ps,
            tc.tile_pool(name="dram", bufs=1, space="DRAM") as dram,
        ):
            eps_c = sb.tile([128, 1], FP, tag="eps_c")
            nc.vector.memset(eps_c, EPS)

            for _ in range(repeats):
                # ---------- input DMAs (all contiguous 2D) ----------
                xle = sb.tile([128, LT, LOWE], BF, tag="xle")
                nc.sync.dma_start(
                    xle[:].rearrange("p i f -> p (i f)"), xle_t[:, :])
                xhtl = sb.tile([128, HC, TPC], BF, tag="xhtl")
                nc.sync.dma_start(
                    xhtl[:].rearrange("p k t -> p (k t)"), xhtl_t[:, :])
                wb = sb.tile([128, WB], BF, tag="wb")
                nc.sync.dma_start(wb[:], wblob[:, :])
                gbp = sb.tile([128, 2 * HC], FP, tag="gbp")
                nc.sync.dma_start(gbp[:], gb_p[:, :])
                thf = wb[:, O_THF:O_PEXT].rearrange("p (k a) -> p k a", k=4)
                pext = wb[:, O_PEXT:O_GWW].rearrange("p (k a) -> p k a", k=3)
                gwwb = wb[:, O_GWW:O_GWWS].rearrange("p (k h) -> p k h", k=2)
                gwws = wb[0:1, O_GWWS:WB]
                gamma_p = gbp[:, 0:HC]
                beta_p = gbp[:, HC:2 * HC]

                # ---------- C Gram: 3 chained groups in one region ----
                ca = ps.tile([128, 2048], FP, tag="big", bufs=2, name="ca")
                for i in range(LT):
                    nc.tensor.matmul(ca[:, 0:LOWE], xle[:, i, 0:128],
                                     xle[:, i, :], start=(i == 0),
                                     stop=(i == LT - 1))
                    nc.tensor.matmul(ca[:, 512:512 + LOWE],
                                     xle[:, i, 128:256], xle[:, i, :],
                                     start=(i == 0), stop=(i == LT - 1))
                    nc.tensor.matmul(ca[0:1, 1024:1024 + LOWE],
                                     xle[:, i, 256:257], xle[:, i, :],
                                     start=(i == 0), stop=(i == LT - 1))
                stg = sb.tile([128, CF], BF, tag="stg")
                nc.vector.tensor_copy(stg[:], ca[:, 0:CF])

                # ---------- theta^T local (overlaps AR latency) -------
                tb = ps.tile([128, 2048], FP, tag="big", bufs=2, name="tb")
                for m in range(LC):
                    for s2 in range(2):
                        q = (m * 2 + s2) * 512
                        for k in range(HC):
                            nc.tensor.matmul(
                                tb[:, q:q + 512],
                                thf[:, k, m * 128:(m + 1) * 128],
                                xhtl[:, k, s2 * 512:(s2 + 1) * 512],
                                start=(k == 0), stop=(k == HC - 1))
                thtl = sb.tile([128, LC, TPC], BF, tag="thtl")
                nc.vector.tensor_copy(
                    thtl[:].rearrange("p m t -> p (m t)"), tb[:])

                # ---------- AllReduce #1 (C, bf16) ----------
                c_in = dram.tile([128, CF], BF, tag="c_in")
                c_out = dram.tile([128, CF], BF, tag="c_out")
                nc.sync.dma_start(c_in[:, :], stg[:])
                if noar:
                    nc.sync.dma_start(c_out[:, :], c_in[:, :])
                else:
                    nc.gpsimd.collective_compute(
                        "AllReduce", mybir.AluOpType.add, replica_groups=rg,
                        ins=[c_in.opt()], outs=[c_out.opt()])
                rstg = sb.tile([128, CF], BF, tag="rstg")
                nc.sync.dma_start(rstg[:], c_out[:, :])
                c0 = rstg[:, 0:LOWE]
                c1 = rstg[:, 512:512 + LOWE]
                srow_g = rstg[0:1, 1024:1024 + LOWE]

                if stage < 3:
                    ztr = sb.tile([128, HC * TPC], FP, tag="ztr")
                    nc.vector.memset(ztr[:, 0:HIGH], 0.0)
                    nc.sync.dma_start(z_out[:, :], ztr[:])
                    continue

                # ---------- T1' = C @ Gww ----------
                def c_lhs(kc, msl):
                    if kc == 0:
                        return c0[:, msl]
                    if kc == 1:
                        return c1[:, msl]
                    return srow_g[0:1, msl]

                t1r = ps.tile([128, 2048], FP, tag="big", bufs=2, name="t1r")
                for m in range(3):
                    msl = (slice(0, 128), slice(128, 256),
                           slice(256, 257))[m]
                    mlen = msl.stop - msl.start
                    dst = t1r[:mlen, m * 512:m * 512 + 512]
                    for k in range(3):
                        klen = 128 if k < 2 else 1
                        rhs = gwwb[:, k, :] if k < 2 else gwws[0:1, :]
                        nc.tensor.matmul(dst, c_lhs(k, msl), rhs,
                                         start=(k == 0), stop=(k == 2))
                t1sb = sb.tile([128, 1536], BF, tag="t1sb")
                nc.vector.tensor_copy(t1sb[:], t1r[:, 0:1536])

                # ---------- V = P_ext^T T1' / BN ----------
                vr = ps.tile([128, 2048], FP, tag="big", bufs=2, name="vr")
                for m in range(LC):
                    dst = vr[:, m * 512:(m + 1) * 512]
                    for k in range(3):
                        klen = 128 if k < 2 else 1
                        rhs = (t1sb[:, k * 512:(k + 1) * 512] if k < 2
                               else t1sb[0:1, 1024:1536])
                        nc.tensor.matmul(dst, pext[:klen, k,
                                                   m * 128:(m + 1) * 128],
                                         rhs, start=(k == 0), stop=(k == 2))
                vsb = sb.tile([128, 1024], BF, tag="vsb")
                nc.vector.tensor_scalar_mul(vsb[:], vr[:, 0:1024], 1.0 / BN)

                # ---------- w_y^T = V^T theta^T (feature-major) -------
                wyt = sb.tile([128, HC, TPC], BF, tag="wyt")
                for half in range(2):
                    wr = ps.tile([128, 2048], FP, tag="big", bufs=2,
                                 name="wr")
                    for hh in range(2):
                        hc = half * 2 + hh
                        for s2 in range(2):
                            q = (hh * 2 + s2) * 512
                            for k in range(LC):
                                nc.tensor.matmul(
                                    wr[:, q:q + 512],
                                    vsb[:, k * 512 + hc * 128:
                                        k * 512 + (hc + 1) * 128],
                                    thtl[:, k, s2 * 512:(s2 + 1) * 512],
                                    start=(k == 0), stop=(k == LC - 1))
                    nc.vector.tensor_copy(
                        wyt[:, half * 2:(half + 1) * 2, :].rearrange(
                            "p h t -> p (h t)"), wr[:])

                # ---------- BN sums + AllReduce #2 ----------
                sqt = sb.tile([128, HC, TPC], FP, tag="sqt")
                nc.vector.tensor_mul(sqt[:], wyt[:], wyt[:])
                st = sb.tile([128, 2 * HC], FP, tag="st")
                nc.vector.reduce_sum(st[:, 0:HC], wyt[:],
                                     axis=mybir.AxisListType.X)
                nc.vector.reduce_sum(st[:, HC:2 * HC], sqt[:],
                                     axis=mybir.AxisListType.X)
                s_in = dram.tile([128, 2 * HC], FP, tag="s_in")
                s_out = dram.tile([128, 2 * HC], FP, tag="s_out")
                nc.sync.dma_start(s_in[:, :], st[:])
                if noar:
                    nc.sync.dma_start(s_out[:, :], s_in[:, :])
                else:
                    nc.gpsimd.collective_compute(
                        "AllReduce", mybir.AluOpType.add, replica_groups=rg,
                        ins=[s_in.opt()], outs=[s_out.opt()])
                sg = sb.tile([128, 2 * HC], FP, tag="sg")
                nc.sync.dma_start(sg[:], s_out[:, :])

                # ---------- stats -> a_p / c_p (per-partition) --------
                msc = sb.tile([128, 2 * HC], FP, tag="msc")
                nc.vector.tensor_scalar_mul(msc[:], sg[:], 1.0 / BN)
                msq = sb.tile([128, HC], FP, tag="msq")
                nc.vector.tensor_mul(msq[:], msc[:, 0:HC], msc[:, 0:HC])
                var_p = sb.tile([128, HC], FP, tag="var_p")
                nc.vector.tensor_sub(var_p[:], msc[:, HC:2 * HC], msq[:])
                std_p = sb.tile([128, HC], FP, tag="std_p")
                nc.scalar.activation(std_p[:], var_p[:],
                                     mybir.ActivationFunctionType.Sqrt,
                                     bias=eps_c[:])
                nc.vector.reciprocal(std_p[:], std_p[:])
                a_p = sb.tile([128, HC], FP, tag="a_p")
                nc.vector.tensor_mul(a_p[:], gamma_p, std_p[:])
                c_p = sb.tile([128, HC], FP, tag="c_p")
                nc.vector.tensor_mul(c_p[:], msc[:, 0:HC], a_p[:])
                nc.vector.tensor_sub(c_p[:], beta_p, c_p[:])

                # ---------- z^T = wy^T*a + c + x_h^T; store -----------
                z_sb = sb.tile([128, HC, TPC], FP, tag="z_sb")
                z1 = sb.tile([128, HC, TPC], FP, tag="z1")
                for hc in range(HC):
                    nc.scalar.activation(
                        z1[:, hc, :], wyt[:, hc, :],
                        mybir.ActivationFunctionType.Identity,
                        bias=c_p[:, hc:hc + 1], scale=a_p[:, hc:hc + 1])
                    nc.vector.tensor_add(z_sb[:, hc, :], z1[:, hc, :],
                                         xhtl[:, hc, :])
                nc.sync.dma_start(
                    z_out[:, :], z_sb[:].rearrange("p k t -> p (k t)"))

    nc.compile()
    return nc


_CACHE: dict[int, "bacc.Bacc"] = {}
_LOCK = threading.Lock()


def _get_nc(repeats: int = 1):
    with _LOCK:
        if repeats not in _CACHE:
            _CACHE[repeats] = build_kernel(repeats)
        return _CACHE[repeats]


def _shard_inputs(inputs: dict) -> list[dict]:
    import ml_dtypes
    bf16 = ml_dtypes.bfloat16
    xh = np.asarray(inputs["x_h"], dtype=np.float32).reshape(BN, HIGH)
    xl = np.asarray(inputs["x_l"], dtype=np.float32).reshape(BN, LOW)
    thw = np.asarray(inputs["theta_w"], np.float32)
    thb = np.asarray(inputs["theta_b"], np.float32)
    gw = np.asarray(inputs["g_w"], np.float32)
    gb = np.asarray(inputs["g_b"], np.float32)
    pw = np.asarray(inputs["phi_w"], np.float32)
    pb = np.asarray(inputs["phi_b"], np.float32)
    ww = np.asarray(inputs["w_w"], np.float32)

    blob = np.zeros((128, WB), np.float32)
    blob[:, O_THF:O_PEXT] = thw.reshape(4, 128, LOW).transpose(
        1, 0, 2).reshape(128, -1)
    pe = np.zeros((128, 3, LOW), np.float32)
    pe[:, 0:2, :] = pw.reshape(2, 128, LOW).transpose(1, 0, 2)
    pe[0, 2, :] = pb
    blob[:, O_PEXT:O_GWW] = pe.reshape(128, -1)
    gww = np.concatenate([gw, gb[None, :]], axis=0) @ ww  # [257, 512]
    blob[:, O_GWW:O_GWWS] = gww[0:256].reshape(2, 128, HIGH).transpose(
        1, 0, 2).reshape(128, -1)
    blob[0, O_GWWS:WB] = gww[256]
    blob_b = np.ascontiguousarray(blob.astype(bf16))

    gbp = np.ascontiguousarray(np.concatenate(
        [np.asarray(inputs["bn_gamma"], np.float32).reshape(HC, 128).T,
         np.asarray(inputs["bn_beta"], np.float32).reshape(HC, 128).T],
        axis=1))
    common = {"wblob": blob_b, "gb_p": gbp}
    maps = []
    for c in range(NCORES):
        tok = slice(c * TPC, (c + 1) * TPC)
        xle = np.concatenate(
            [xl[tok].reshape(LT, 128, LOW),
             np.ones((LT, 128, 1), np.float32)], axis=2)
        xle_t = xle.transpose(1, 0, 2).reshape(128, -1).astype(bf16)
        xhtl_t = xh[tok].T.reshape(HC, 128, TPC).transpose(
            1, 0, 2).reshape(128, -1).astype(bf16)
        maps.append({
            "xle_t": np.ascontiguousarray(xle_t),
            "xhtl_t": np.ascontiguousarray(xhtl_t),
            **common,
        })
    return maps


def kernel(**inputs) -> np.ndarray:
    nc = _get_nc(1)
    in_maps = _shard_inputs(inputs)
    res = run_bass_kernel_spmd(nc, in_maps, list(range(NCORES)))
    zs = []
    for c in range(NCORES):
        zt = res.results[c]["z"].reshape(128, HC, TPC)
        zs.append(zt.transpose(1, 0, 2).reshape(HIGH, TPC).T)
    return np.concatenate(zs, axis=0).reshape(B, N, HIGH)


# revision 18
# speedup vs baseline: 12.0778x; 1.0087x over previous
"""MFA block kernel for 8 Trainium2 NeuronCores.

Full (unsharded) inputs in, full output out.

v4: tokens sharded 1024/core, TWO small AllReduces, minimal op count.
The execution stack here prices ops roughly as: matmul ~90us each
(shape-insensitive), vector op ~170us FIXED with near-free elements,
contiguous 2D DMA ~fast, strided DMA ~1ms/MB, AllReduce ~1-2ms.  So the
kernel minimizes op COUNT: Gram/matmul outputs are packed into large
PSUM regions drained by ONE big vector op each, stats use single huge
reduces, and all host-visible layouts are pre-tiled so every DMA is
per-partition contiguous.

Pipeline (everything bf16 except stats/residual math):
  C = X_ext^T X_ext (rect + s-row, 3 chained MM groups) -> AllReduce#1
  theta^T = Th^T x_h^T + thb  (feature-major, from host-shipped x_h^T)
  M' = P_ext^T C G_ext / BN;  V = M' @ w_w
  w_y^T = V^T theta^T         (feature-major)
  BN sums via one mul + two reduces -> AllReduce#2 ([128,8] f32)
  z^T = (w_y^T)*a + c + x_h^T   (a,c per-partition scalars), one DMA out
w_b is dropped (BN cancels constant shifts).
"""

import threading

import numpy as np

import concourse.tile as tile
from concourse import bacc, mybir
from concourse.bass_utils import run_bass_kernel_spmd

FP = mybir.dt.float32
BF = mybir.dt.bfloat16
HIGH = 512
LOW = 256
B = 8
N = 1024
BN = B * N
NCORES = 8
TPC = BN // NCORES    # 1024
LT = TPC // 128       # 8
HC = HIGH // 128      # 4
LC = LOW // 128       # 2
EPS = 1e-5
LOWE = LOW + 1        # 257

# weight blob column offsets (bf16, [128, WB])
O_THF = 0                      # [4, 256] theta_w (high-major)
O_PEXT = O_THF + 4 * 256       # [3, 256] P_ext chunks (row 256 on p0)
O_GWW = O_PEXT + 3 * 256       # [2, 512] Gww = G_ext @ w_w rows 0:512
O_GWWS = O_GWW + 2 * 512       # [1, 512] Gww row 256 (on p0)
WB = O_GWWS + 512

# C-region packing (psum cols, f32): m0 rows at 0, m1 rows at 512,
# s-row at 1024 (bank-aligned); payload ships cols 0:1536 raw.
CF = 1536

# single packed bf16 input: [xle | xhtl | wblob]
I_XLE = 0
I_XHTL = I_XLE + 8 * 257       # 2056
I_WB = I_XHTL + 4 * 1024       # 6152
XIN = I_WB + WB

rg = [list(range(NCORES))]


def build_kernel(repeats: int = 1, noar: bool = False, stage: int = 9):
    nc = bacc.Bacc("TRN2", target_bir_lowering=False, debug=False,
                   num_devices=NCORES)

    xin_d = nc.declare_dram_parameter("xin_d", [128, XIN], BF,
                                      isOutput=False)
    gb_p = nc.declare_dram_parameter("gb_p", [128, 2 * HC], FP,
                                     isOutput=False)
    z_out = nc.declare_dram_parameter("z", [128, HC * TPC], FP,
                                      isOutput=True)

    with tile.TileContext(nc) as tc:
        with (
            tc.tile_pool(name="sb", bufs=1) as sb,
            tc.tile_pool(name="ps", bufs=1, space="PSUM") as ps,
            tc.tile_pool(name="dram", bufs=1, space="DRAM") as dram,
        ):
            eps_c = sb.tile([128, 1], FP, tag="eps_c")
            nc.vector.memset(eps_c, EPS)

            for _ in range(repeats):
                # ---------- input DMAs (one packed bf16 + f32 rows) ---
                xin = sb.tile([128, XIN], BF, tag="xin")
                nc.sync.dma_start(xin[:], xin_d[:, :])
                gbp = sb.tile([128, 2 * HC], FP, tag="gbp")
                nc.sync.dma_start(gbp[:], gb_p[:, :])
                xle = xin[:, I_XLE:I_XHTL].rearrange("p (i f) -> p i f",
                                                     i=LT)
                xhtl = xin[:, I_XHTL:I_WB].rearrange("p (k t) -> p k t",
                                                     k=HC)
                wb = xin[:, I_WB:XIN]
                thf = wb[:, O_THF:O_PEXT].rearrange("p (k a) -> p k a", k=4)
                pext = wb[:, O_PEXT:O_GWW].rearrange("p (k a) -> p k a", k=3)
                gwwb = wb[:, O_GWW:O_GWWS].rearrange("p (k h) -> p k h", k=2)
                gwws = wb[0:1, O_GWWS:WB]
                gamma_p = gbp[:, 0:HC]
                beta_p = gbp[:, HC:2 * HC]

                # ---------- C Gram: 3 chained groups in one region ----
                ca = ps.tile([128, 2048], FP, tag="big", bufs=2, name="ca")
                for i in range(LT):
                    nc.tensor.matmul(ca[:, 0:LOWE], xle[:, i, 0:128],
                                     xle[:, i, :], start=(i == 0),
                                     stop=(i == LT - 1))
                    nc.tensor.matmul(ca[:, 512:512 + LOWE],
                                     xle[:, i, 128:256], xle[:, i, :],
                                     start=(i == 0), stop=(i == LT - 1))
                    nc.tensor.matmul(ca[0:1, 1024:1024 + LOWE],
                                     xle[:, i, 256:257], xle[:, i, :],
                                     start=(i == 0), stop=(i == LT - 1))
                stg = sb.tile([128, CF], BF, tag="stg")
                nc.vector.tensor_copy(stg[:], ca[:, 0:CF])

                # ---------- theta^T local (overlaps AR latency) -------
                tb = ps.tile([128, 2048], FP, tag="big", bufs=2, name="tb")
                for m in range(LC):
                    for s2 in range(2):
                        q = (m * 2 + s2) * 512
                        for k in range(HC):
                            nc.tensor.matmul(
                                tb[:, q:q + 512],
                                thf[:, k, m * 128:(m + 1) * 128],
                                xhtl[:, k, s2 * 512:(s2 + 1) * 512],
                                start=(k == 0), stop=(k == HC - 1))
                thtl = sb.tile([128, LC, TPC], BF, tag="thtl")
                nc.vector.tensor_copy(
                    thtl[:].rearrange("p m t -> p (m t)"), tb[:])

                # ---------- AllReduce #1 (C, bf16) ----------
                c_in = dram.tile([128, CF], BF, tag="c_in")
                c_out = dram.tile([128, CF], BF, tag="c_out")
                nc.sync.dma_start(c_in[:, :], stg[:])
                if noar:
                    nc.sync.dma_start(c_out[:, :], c_in[:, :])
                else:
                    nc.gpsimd.collective_compute(
                        "AllReduce", mybir.AluOpType.add, replica_groups=rg,
                        ins=[c_in.opt()], outs=[c_out.opt()])
                rstg = sb.tile([128, CF], BF, tag="rstg")
                nc.sync.dma_start(rstg[:], c_out[:, :])
                c0 = rstg[:, 0:LOWE]
                c1 = rstg[:, 512:512 + LOWE]
                srow_g = rstg[0:1, 1024:1024 + LOWE]

                if stage < 3:
                    ztr = sb.tile([128, HC * TPC], FP, tag="ztr")
                    nc.vector.memset(ztr[:, 0:HIGH], 0.0)
                    nc.sync.dma_start(z_out[:, :], ztr[:])
                    continue

                # ---------- T1' = C @ Gww ----------
                def c_lhs(kc, msl):
                    if kc == 0:
                        return c0[:, msl]
                    if kc == 1:
                        return c1[:, msl]
                    return srow_g[0:1, msl]

                t1r = ps.tile([128, 2048], FP, tag="big", bufs=2, name="t1r")
                for m in range(3):
                    msl = (slice(0, 128), slice(128, 256),
                           slice(256, 257))[m]
                    mlen = msl.stop - msl.start
                    dst = t1r[:mlen, m * 512:m * 512 + 512]
                    for k in range(3):
                        klen = 128 if k < 2 else 1
                        rhs = gwwb[:, k, :] if k < 2 else gwws[0:1, :]
                        nc.tensor.matmul(dst, c_lhs(k, msl), rhs,
                                         start=(k == 0), stop=(k == 2))
                t1sb = sb.tile([128, 1536], BF, tag="t1sb")
                nc.vector.tensor_copy(t1sb[:], t1r[:, 0:1536])

                # ---------- V = P_ext^T T1' / BN ----------
                vr = ps.tile([128, 2048], FP, tag="big", bufs=2, name="vr")
                for m in range(LC):
                    dst = vr[:, m * 512:(m + 1) * 512]
                    for k in range(3):
                        klen = 128 if k < 2 else 1
                        rhs = (t1sb[:, k * 512:(k + 1) * 512] if k < 2
                               else t1sb[0:1, 1024:1536])
                        nc.tensor.matmul(dst, pext[:klen, k,
                                                   m * 128:(m + 1) * 128],
                                         rhs, start=(k == 0), stop=(k == 2))
                vsb = sb.tile([128, 1024], BF, tag="vsb")
                nc.vector.tensor_scalar_mul(vsb[:], vr[:, 0:1024], 1.0 / BN)

                # ---------- w_y^T = V^T theta^T (feature-major) -------
                wyt = sb.tile([128, HC, TPC], BF, tag="wyt")
                for half in range(2):
                    wr = ps.tile([128, 2048], FP, tag="big", bufs=2,
                                 name="wr")
                    for hh in range(2):
                        hc = half * 2 + hh
                        for s2 in range(2):
                            q = (hh * 2 + s2) * 512
                            for k in range(LC):
                                nc.tensor.matmul(
                                    wr[:, q:q + 512],
                                    vsb[:, k * 512 + hc * 128:
                                        k * 512 + (hc + 1) * 128],
                                    thtl[:, k, s2 * 512:(s2 + 1) * 512],
                                    start=(k == 0), stop=(k == LC - 1))
                    nc.vector.tensor_copy(
                        wyt[:, half * 2:(half + 1) * 2, :].rearrange(
                            "p h t -> p (h t)"), wr[:])

                # ---------- BN sums + AllReduce #2 ----------
                sqt = sb.tile([128, HC, TPC], FP, tag="sqt")
                nc.vector.tensor_mul(sqt[:], wyt[:], wyt[:])
                st = sb.tile([128, 2 * HC], FP, tag="st")
                nc.vector.reduce_sum(st[:, 0:HC], wyt[:],
                                     axis=mybir.AxisListType.X)
                nc.vector.reduce_sum(st[:, HC:2 * HC], sqt[:],
                                     axis=mybir.AxisListType.X)
                s_in = dram.tile([128, 2 * HC], FP, tag="s_in")
                s_out = dram.tile([128, 2 * HC], FP, tag="s_out")
                nc.sync.dma_start(s_in[:, :], st[:])
                if noar:
                    nc.sync.dma_start(s_out[:, :], s_in[:, :])
                else:
                    nc.gpsimd.collective_compute(
                        "AllReduce", mybir.AluOpType.add, replica_groups=rg,
                        ins=[s_in.opt()], outs=[s_out.opt()])
                sg = sb.tile([128, 2 * HC], FP, tag="sg")
                nc.sync.dma_start(sg[:], s_out[:, :])

                # ---------- stats -> a_p / c_p (per-partition) --------
                msc = sb.tile([128, 2 * HC], FP, tag="msc")
                nc.vector.tensor_scalar_mul(msc[:], sg[:], 1.0 / BN)
                msq = sb.tile([128, HC], FP, tag="msq")
                nc.vector.tensor_mul(msq[:], msc[:, 0:HC], msc[:, 0:HC])
                var_p = sb.tile([128, HC], FP, tag="var_p")
                nc.vector.tensor_sub(var_p[:], msc[:, HC:2 * HC], msq[:])
                std_p = sb.tile([128, HC], FP, tag="std_p")
                nc.scalar.activation(std_p[:], var_p[:],
                                     mybir.ActivationFunctionType.Sqrt,
                                     bias=eps_c[:])
                nc.vector.reciprocal(std_p[:], std_p[:])
                a_p = sb.tile([128, HC], FP, tag="a_p")
                nc.vector.tensor_mul(a_p[:], gamma_p, std_p[:])
                c_p = sb.tile([128, HC], FP, tag="c_p")
                nc.vector.tensor_mul(c_p[:], msc[:, 0:HC], a_p[:])
                nc.vector.tensor_sub(c_p[:], beta_p, c_p[:])

                # ---------- z^T = wy^T*a + c + x_h^T; store -----------
                z_sb = sb.tile([128, HC, TPC], FP, tag="z_sb")
                z1 = sb.tile([128, HC, TPC], FP, tag="z1")
                for hc in range(HC):
                    nc.scalar.activation(
                        z1[:, hc, :], wyt[:, hc, :],
                        mybir.ActivationFunctionType.Identity,
                        bias=c_p[:, hc:hc + 1], scale=a_p[:, hc:hc + 1])
                    nc.vector.tensor_add(z_sb[:, hc, :], z1[:, hc, :],
                                         xhtl[:, hc, :])
                nc.sync.dma_start(
                    z_out[:, :], z_sb[:].rearrange("p k t -> p (k t)"))

    nc.compile()
    return nc


_CACHE: dict[int, "bacc.Bacc"] = {}
_LOCK = threading.Lock()


def _get_nc(repeats: int = 1):
    with _LOCK:
        if repeats not in _CACHE:
            _CACHE[repeats] = build_kernel(repeats)
        return _CACHE[repeats]


def _shard_inputs(inputs: dict) -> list[dict]:
    import ml_dtypes
    bf16 = ml_dtypes.bfloat16
    xh = np.asarray(inputs["x_h"], dtype=np.float32).reshape(BN, HIGH)
    xl = np.asarray(inputs["x_l"], dtype=np.float32).reshape(BN, LOW)
    thw = np.asarray(inputs["theta_w"], np.float32)
    thb = np.asarray(inputs["theta_b"], np.float32)
    gw = np.asarray(inputs["g_w"], np.float32)
    gb = np.asarray(inputs["g_b"], np.float32)
    pw = np.asarray(inputs["phi_w"], np.float32)
    pb = np.asarray(inputs["phi_b"], np.float32)
    ww = np.asarray(inputs["w_w"], np.float32)

    blob = np.zeros((128, WB), np.float32)
    blob[:, O_THF:O_PEXT] = thw.reshape(4, 128, LOW).transpose(
        1, 0, 2).reshape(128, -1)
    pe = np.zeros((128, 3, LOW), np.float32)
    pe[:, 0:2, :] = pw.reshape(2, 128, LOW).transpose(1, 0, 2)
    pe[0, 2, :] = pb
    blob[:, O_PEXT:O_GWW] = pe.reshape(128, -1)
    gww = np.concatenate([gw, gb[None, :]], axis=0) @ ww  # [257, 512]
    blob[:, O_GWW:O_GWWS] = gww[0:256].reshape(2, 128, HIGH).transpose(
        1, 0, 2).reshape(128, -1)
    blob[0, O_GWWS:WB] = gww[256]
    blob_b = np.ascontiguousarray(blob.astype(bf16))

    gbp = np.ascontiguousarray(np.concatenate(
        [np.asarray(inputs["bn_gamma"], np.float32).reshape(HC, 128).T,
         np.asarray(inputs["bn_beta"], np.float32).reshape(HC, 128).T],
        axis=1))
    common = {"gb_p": gbp}
    maps = []
    for c in range(NCORES):
        tok = slice(c * TPC, (c + 1) * TPC)
        xle = np.concatenate(
            [xl[tok].reshape(LT, 128, LOW),
             np.ones((LT, 128, 1), np.float32)], axis=2)
        xle_t = xle.transpose(1, 0, 2).reshape(128, -1).astype(bf16)
        xhtl_t = xh[tok].T.reshape(HC, 128, TPC).transpose(
            1, 0, 2).reshape(128, -1).astype(bf16)
        xin = np.concatenate([xle_t, xhtl_t, blob_b], axis=1)
        maps.append({
            "xin_d": np.ascontiguousarray(xin),
            **common,
        })
    return maps


def kernel(**inputs) -> np.ndarray:
    nc = _get_nc(1)
    in_maps = _shard_inputs(inputs)
    res = run_bass_kernel_spmd(nc, in_maps, list(range(NCORES)))
    zs = []
    for c in range(NCORES):
        zt = res.results[c]["z"].reshape(128, HC, TPC)
        zs.append(zt.transpose(1, 0, 2).reshape(HIGH, TPC).T)
    return np.concatenate(zs, axis=0).reshape(B, N, HIGH)
